# revision 24
# baseline (speedup 1.0000x reference)
"""Multi-head attention (B=2, S=2048, D=1024, H=16) on 8 TRN2 NeuronCores.

Sharding: batch x head-group. Core c handles batch b=c//4 and heads
[4g, 4g+4) with g=c%4 (column-parallel QKV projections, row-parallel
output projection). Each core emits a partial [S, D] output; the host
sums the 4 partials per batch (the row-parallel all-reduce).

Chunk-major pipeline (v2): the sequence is processed in 4 chunks of 512
queries. Per chunk: project q/k for that chunk (+v for its key blocks),
run both head-pairs' scores->exp->AV, normalize, and do the output
projection — so softmax normalization and the out-projection of chunk c
hide under the attention of chunk c+1 instead of forming a serial tail.
Host re-layouts q/k/v so every chunk's input is one contiguous DMA.

Other key choices (all matmuls bf16 with f32 PSUM accum):
- Scores are computed transposed (scoresT[k, q]); the K=64 head-pair
  matmuls run concurrently in the PE array via base-partition row
  tiling. vh carries a ones column so attention row-sums fall out of
  the AV matmul.
- Causal masking of diagonal blocks is a PE preload: a [128,128]
  additive -1e9 matmul with start=True, which the score matmul then
  accumulates onto (cols beyond 128 are overwritten since their
  has_written bits stay clear) — no VectorE masking pass.
- Softmax renormalization: row sums gathered to a [4,512] tile, 1/x on
  ScalarE as exp(-ln(x)) (both fns in one ACT table set), broadcast
  across partitions by a tiny one-hot fp16 matmul, applied by VectorE.
- ~80 junk warmup matmuls at t=0 keep the PE HAM clock-gate warm while
  the first input DMAs land.
"""

import os
import numpy as np
import ml_dtypes

import concourse.bass as bass
import concourse.tile as tile
from concourse import bacc, mybir
from concourse.bass_utils import run_bass_kernel_spmd

B, S, D, H = 2, 2048, 1024, 16
HD = D // H          # 64
HL = H // 4          # 4 heads per core
PL = HL * HD         # 256 local projection dim
KT = D // 128        # 8 contraction blocks
SB = S // 128        # 16 sequence blocks of 128
CH = S // 512        # 4 sequence chunks of 512
F32 = mybir.dt.float32
F16 = mybir.dt.float16
DT = mybir.dt.bfloat16
NP_DT = ml_dtypes.bfloat16
USE_ACT_RECIP = False  # Ln+Exp thrash ACT table sets (9 loads); DVE recip hides under the skew

_cache = {}
last_results = None


def build_program():
    if "nc" in _cache:
        return _cache["nc"]
    nc = bacc.Bacc("TRN2", target_bir_lowering=False, debug=False, num_devices=8)

    # inputs, host-relaid so every DMA is contiguous with >=2KB/partition
    qc_d = nc.dram_tensor("qc", [CH, 128, KT, 512], DT, kind="ExternalInput")
    kc_d = nc.dram_tensor("kc", [CH, 128, KT, 512], DT, kind="ExternalInput")
    vc_d = nc.dram_tensor("vc", [SB, 128, KT, 128], DT, kind="ExternalInput")
    wq_d = nc.dram_tensor("wq", [128, KT, PL], DT, kind="ExternalInput")
    wk_d = nc.dram_tensor("wk", [128, KT, PL], DT, kind="ExternalInput")
    wv_d = nc.dram_tensor("wv", [128, KT, PL], DT, kind="ExternalInput")
    wf_d = nc.dram_tensor("wf", [128, 2, D], DT, kind="ExternalInput")
    bq_d = nc.dram_tensor("bq2", [128, 2], F32, kind="ExternalInput")
    bk_d = nc.dram_tensor("bk2", [128, 2], F32, kind="ExternalInput")
    bv_d = nc.dram_tensor("bv1", [1, PL], F32, kind="ExternalInput")
    bf_d = nc.dram_tensor("bf1", [1, D], F32, kind="ExternalInput")
    tri_d = nc.dram_tensor("tri", [128, 128], DT, kind="ExternalInput")
    idn_d = nc.dram_tensor("idn", [128, 128], DT, kind="ExternalInput")
    sel_d = nc.dram_tensor("sel", [4, 4 * HD], F16, kind="ExternalInput")
    out_d = nc.dram_tensor("out", [S, D], DT, kind="ExternalOutput")

    ADD = mybir.AluOpType.add
    MUL = mybir.AluOpType.mult
    EXP = mybir.ActivationFunctionType.Exp
    LN = mybir.ActivationFunctionType.Ln

    with tile.TileContext(nc) as tc:
        with (
            tc.tile_pool(name="singles", bufs=1) as singles,
            tc.tile_pool(name="qk", bufs=3) as qkp,
            tc.tile_pool(name="vin", bufs=6) as vinp,
            tc.tile_pool(name="epool", bufs=6) as epool,
            tc.tile_pool(name="apool", bufs=6) as apool,
            tc.tile_pool(name="npool", bufs=2) as npool,
            tc.tile_pool(name="opool", bufs=2) as opool,
            tc.tile_pool(name="psum", bufs=2, space="PSUM") as psum,
        ):
            wq_sb = singles.tile([128, KT, PL], DT)
            wk_sb = singles.tile([128, KT, PL], DT)
            wv_sb = singles.tile([128, KT, PL], DT)
            wf_sb = singles.tile([128, 2, D], DT)
            bq_sb = singles.tile([128, 2], F32)
            bk_sb = singles.tile([128, 2], F32)
            bv_bc = singles.tile([128, PL], F32)
            bf_bc = singles.tile([128, D], F32)
            tri_sb = singles.tile([128, 128], DT)
            idn_sb = singles.tile([128, 128], DT)
            sel_sb = singles.tile([4, 4 * HD], F16)

            qhT = singles.tile([128, 2, S], DT)   # [p within pair, pair, s]
            khT = singles.tile([128, 2, S], DT)
            vh = singles.tile([128, SB, HL, HD + 1], DT)  # [s in blk, sblk, h, hd|1]
            xn = singles.tile([128, 2, S], DT)    # normalized attn out
            nc.vector.memset(vh[:, :, :, HD : HD + 1], 1.0)

            # junk matmuls keep the PE HAM activity window busy across
            # dependency stalls (initial DMA wait, hb boundaries, the
            # epilogue's reciprocal chain) so real matmuls run at 2.4 GHz
            wu = singles.tile([128, 128], DT)
            nc.vector.memset(wu, 0.0)

            def junk(n):
                for _ in range(n):
                    wp = psum.tile([128, 128], F32, tag="C", bufs=2, name="wu")
                    nc.tensor.matmul(wp, lhsT=wu, rhs=wu, start=True, stop=True)

            junk(80)

            # weights first (small), then per-chunk inputs
            nc.sync.dma_start(wq_sb, wq_d.ap())
            nc.sync.dma_start(bq_sb, bq_d.ap())
            nc.sync.dma_start(wk_sb, wk_d.ap())
            nc.sync.dma_start(bk_sb, bk_d.ap())
            nc.sync.dma_start(wv_sb, wv_d.ap())
            nc.sync.dma_start(tri_sb, tri_d.ap())
            nc.sync.dma_start(idn_sb, idn_d.ap())
            nc.sync.dma_start(bv_bc, bv_d.ap().to_broadcast([128, PL]))

            # DMA issue is decoupled from compute: inputs for chunk c+1 are
            # DMA'd at the start of chunk c, and their projections are
            # emitted as PE "filler" between attention blocks of chunk c
            # (the PE queue is strictly in-order, so without filler it
            # would idle every block waiting on ScalarE's exp).
            def issue_qk(c):
                xq = qkp.tile([128, KT, 512], DT, tag="xin")
                nc.sync.dma_start(xq, qc_d.ap()[c])
                xk = qkp.tile([128, KT, 512], DT, tag="xin")
                nc.sync.dma_start(xk, kc_d.ap()[c])
                return xq, xk

            def issue_v(c):
                vts = []
                for sb in range(4 * c, 4 * c + 4):
                    vt = vinp.tile([128, KT, 128], DT, tag="vin", bufs=8)
                    nc.sync.dma_start(vt, vc_d.ap()[sb])
                    vts.append(vt)
                return vts

            def proj_item(xt, c, pt, w_sb, b_sb, out_sb):
                pp = psum.tile([128, 512], F32, tag="C", bufs=2, name="pp")
                for kk in range(KT):
                    nc.tensor.matmul(
                        pp,
                        lhsT=w_sb[:, kk, 128 * pt : 128 * (pt + 1)],
                        rhs=xt[:, kk, :],
                        start=(kk == 0),
                        stop=(kk == KT - 1),
                    )
                nc.vector.tensor_scalar_add(
                    out_sb[:, pt, 512 * c : 512 * (c + 1)],
                    pp,
                    b_sb[:, pt : pt + 1],
                )

            def vproj_item(vt, sb):
                pv = psum.tile([128, PL], F32, tag="C", bufs=2, name="pv")
                for kk in range(KT):
                    nc.tensor.matmul(
                        pv,
                        lhsT=vt[:, kk, :],
                        rhs=wv_sb[:, kk, :],
                        start=(kk == 0),
                        stop=(kk == KT - 1),
                    )
                nc.vector.tensor_tensor(
                    out=vh[:, sb, :, 0:HD],
                    in0=pv.rearrange("p (h e) -> p h e", h=HL),
                    in1=bv_bc.rearrange("p (h e) -> p h e", h=HL),
                    op=ADD,
                )

            def norm_item(c, r, xas, rec16, krows=4):
                # rec16 has `krows` partitions; row r % krows holds head r's
                # 1/rowsum. sel's one-hot layout makes any [krows, 64] slice
                # at column 64r a valid broadcast selector.
                hb, par = r // 2, r % 2
                rb = psum.tile([HD, 512], F32, tag="C", bufs=2, name="rb")
                nc.tensor.matmul(
                    rb,
                    lhsT=sel_sb[0:krows, HD * (r % krows) : HD * (r % krows + 1)],
                    rhs=rec16,
                    start=True,
                    stop=True,
                )
                xt_n = npool.tile([HD, 512], DT, tag="xtn", bufs=4)
                nc.vector.tensor_tensor(out=xt_n, in0=xas[r][0:HD, :], in1=rb, op=MUL)
                nc.sync.dma_start(
                    xn[64 * par : 64 * par + 64, hb, 512 * c : 512 * (c + 1)], xt_n
                )

            def d_item(c, ib, split_dma=False):
                ob = opool.tile([128, D], DT, tag="ob")
                for oc in range(2):
                    po = psum.tile([128, 512], F32, tag="C", bufs=2, name="po")
                    for t in range(2):
                        nc.tensor.matmul(
                            po,
                            lhsT=xn[:, t, 128 * ib : 128 * (ib + 1)],
                            rhs=wf_sb[:, t, 512 * oc : 512 * (oc + 1)],
                            start=(t == 0),
                            stop=(t == 1),
                        )
                    nc.vector.tensor_tensor(
                        out=ob[:, 512 * oc : 512 * (oc + 1)],
                        in0=po,
                        in1=bf_bc[:, 512 * oc : 512 * (oc + 1)],
                        op=ADD,
                    )
                    if split_dma:
                        nc.sync.dma_start(
                            out_d.ap()[
                                128 * ib : 128 * (ib + 1),
                                512 * oc : 512 * (oc + 1),
                            ],
                            ob[:, 512 * oc : 512 * (oc + 1)],
                        )
                if not split_dma:
                    nc.sync.dma_start(out_d.ap()[128 * ib : 128 * (ib + 1), :], ob)

            def attn_chunk(c, filler, split_norm=False):
                """Attention for chunk c, interleaving `filler` PE work into
                the exp-latency gaps. AV(bj) is emitted one block late so
                scores(bj+1)+filler cover exp(bj)'s ScalarE latency.
                split_norm (last chunk): reciprocal runs per head-pair so
                hb0's normalize joins the filler during hb1's attention."""
                xas = {}
                recs = {}
                sums = npool.tile([4, 512], F32, tag="sums", bufs=3)
                nbj = 4 * c + 4
                slots = 2 * nbj

                def pop_filler(n=None):
                    # floor pacing leaves a remainder that flushes at the
                    # hb/chunk boundaries — exactly where the PE would
                    # otherwise idle on psp-slot waits and HAM rethrottles
                    nonlocal slots
                    if n is None:
                        n = len(filler) // slots if slots > 0 else 0
                    for _ in range(min(n, len(filler))):
                        filler.pop(0)()
                    slots = max(slots - 1, 0)

                for hb in range(2):
                    pxs = [
                        psum.tile([128, 512], F32, tag="B", bufs=2, name=f"px{p_}")
                        for p_ in range(2)
                    ]
                    pend_av = None
                    for bj in range(nbj):
                        band = bj >= 4 * c
                        i0 = 128 * bj if band else 512 * c
                        w = 512 * (c + 1) - i0
                        o = i0 - 512 * c
                        psp = psum.tile([128, 2, 512], F32, tag="A", bufs=2)
                        for par in range(2):
                            nc.tensor.matmul(
                                psp[:, par, 0:w],
                                lhsT=khT[
                                    64 * par : 64 * par + 64,
                                    hb,
                                    128 * bj : 128 * (bj + 1),
                                ],
                                rhs=qhT[64 * par : 64 * par + 64, hb, i0 : i0 + w],
                                start=True,
                                stop=not band,
                            )
                        if band:
                            # causal masking on the PE: accumulate a -1e9
                            # upper-triangle matmul onto the diagonal
                            # 128x128 sub-block (cols 0:128)
                            for par in range(2):
                                nc.tensor.matmul(
                                    psp[:, par, 0:128],
                                    lhsT=tri_sb,
                                    rhs=idn_sb,
                                    start=False,
                                    stop=True,
                                )
                        et = epool.tile([128, 2, 512], DT, tag="et")
                        nc.scalar.activation(et[:, :, 0:w], psp[:, :, 0:w], EXP)
                        if pend_av is not None:
                            pop_filler()
                            pend_av()
                        pend_av = (
                            lambda et=et, w=w, o=o, bj=bj: [
                                nc.tensor.matmul(
                                    pxs[par][0 : HD + 1, o : o + w],
                                    lhsT=vh[:, bj, 2 * hb + par, :],
                                    rhs=et[:, par, 0:w],
                                    start=(bj == 0),
                                    stop=(bj == nbj - 1),
                                )
                                for par in range(2)
                            ]
                        )
                    pop_filler()
                    junk(6)  # cover the last exp's lag at the hb boundary
                    pend_av()
                    pop_filler(2)  # boundary flush: cover the psp-slot wait
                    junk(4)
                    if split_norm:
                        sums_h = npool.tile([2, 512], F32, tag="sums2", name="sums_h")
                    else:
                        sums_h = sums
                    for par in range(2):
                        r = 2 * hb + par
                        xa = apool.tile([HD + 1, 512], F32, tag="xa", bufs=10)
                        nc.vector.tensor_copy(out=xa, in_=pxs[par][0 : HD + 1, :])
                        row = par if split_norm else r
                        nc.sync.dma_start(sums_h[row : row + 1, :], xa[HD : HD + 1, :])
                        xas[r] = xa
                    if split_norm:
                        if hb == 0:
                            rec = npool.tile([2, 512], F32, tag="rec2")
                            nc.vector.reciprocal(rec, sums_h)
                            r16 = npool.tile([2, 512], F16, tag="rec16b")
                            nc.vector.tensor_copy(r16, rec)
                            for r in range(2):
                                filler.append(
                                    lambda r=r, r16=r16: norm_item(
                                        c, r, xas, r16, krows=2
                                    )
                                )
                        else:
                            recs[1] = sums_h  # epilogue pipelines the recip
                if split_norm:
                    # drain leftover filler (hb0's normalize rides here)
                    for f in filler:
                        f()
                    return xas, recs[1]
                rec16 = npool.tile([4, 512], F16, tag="rec16")
                rec = npool.tile([4, 512], F32, tag="rec")
                nc.vector.reciprocal(rec, sums)
                nc.vector.tensor_copy(rec16, rec)
                # drain any leftover filler
                for f in filler:
                    f()
                return xas, rec16

            # ---- prologue: chunk 0 inputs + projections, serial ----
            xq0, xk0 = issue_qk(0)
            vts0 = issue_v(0)
            for pt in range(2):
                proj_item(xq0, 0, pt, wq_sb, bq_sb, qhT)
            for pt in range(2):
                proj_item(xk0, 0, pt, wk_sb, bk_sb, khT)
            for j, sb in enumerate(range(0, 4)):
                vproj_item(vts0[j], sb)

            pending = None
            for c in range(CH):
                filler = []
                if c + 1 < CH:
                    xq, xk = issue_qk(c + 1)
                    vts = issue_v(c + 1)
                    if c == 0:
                        nc.sync.dma_start(wf_sb, wf_d.ap())
                        nc.sync.dma_start(bf_bc, bf_d.ap().to_broadcast([128, D]))
                        nc.sync.dma_start(sel_sb, sel_d.ap())
                if pending is not None:
                    pc, pxas, prec16 = pending
                    for r in range(4):
                        filler.append(
                            lambda r=r: norm_item(pc, r, pxas, prec16)
                        )
                if c + 1 < CH:
                    for pt in range(2):
                        filler.append(
                            lambda pt=pt, xq=xq, cn=c + 1: proj_item(
                                xq, cn, pt, wq_sb, bq_sb, qhT
                            )
                        )
                    for pt in range(2):
                        filler.append(
                            lambda pt=pt, xk=xk, cn=c + 1: proj_item(
                                xk, cn, pt, wk_sb, bk_sb, khT
                            )
                        )
                if pending is not None:
                    pc = pending[0]
                    for ib in range(4 * pc, 4 * pc + 4):
                        filler.append(lambda ib=ib, pc=pc: d_item(pc, ib))
                if c + 1 < CH:
                    for j, sb in enumerate(range(4 * (c + 1), 4 * (c + 1) + 4)):
                        filler.append(
                            lambda j=j, sb=sb, vts=vts: vproj_item(vts[j], sb)
                        )
                state = attn_chunk(c, filler, split_norm=(c == CH - 1))
                pending = (c, *state)

            # ---- epilogue: last chunk's pair-1 normalize + out-projection,
            # pipelined in 128-query column stages so the DVE reciprocal
            # overlaps the selector/scale/projection of earlier columns.
            # (pair 0 was normalized as filler during the last chunk's hb1.)
            junk(40)  # keep HAM warm while the first reciprocal resolves
            pc, pxas, sums_h1 = pending
            for qb in range(4):
                cs = slice(128 * qb, 128 * (qb + 1))
                rq = npool.tile([2, 128], F32, tag="recq")
                nc.vector.reciprocal(rq, sums_h1[:, cs])
                r16q = npool.tile([2, 128], F16, tag="recq16")
                nc.vector.tensor_copy(r16q, rq)
                for r in (2, 3):
                    par = r % 2
                    rb = psum.tile([HD, 128], F32, tag="C", bufs=2, name="rbq")
                    nc.tensor.matmul(
                        rb,
                        lhsT=sel_sb[0:2, HD * par : HD * (par + 1)],
                        rhs=r16q,
                        start=True,
                        stop=True,
                    )
                    xt_n = npool.tile([HD, 128], DT, tag="xtnq", bufs=4)
                    nc.vector.tensor_tensor(
                        out=xt_n, in0=pxas[r][0:HD, cs], in1=rb, op=MUL
                    )
                    nc.sync.dma_start(
                        xn[
                            64 * par : 64 * par + 64,
                            1,
                            512 * pc + 128 * qb : 512 * pc + 128 * (qb + 1),
                        ],
                        xt_n,
                    )
                d_item(pc, 4 * pc + qb, split_dma=(qb == 3))

    nc.compile()
    _cache["nc"] = nc
    return nc


def _wlayout(wT):
    # [D, PL] -> SBUF layout [128, KT, PL]
    return np.ascontiguousarray(wT.reshape(KT, 128, PL).transpose(1, 0, 2)).astype(NP_DT)


def _flayout(wT):
    # [PL, D] -> SBUF layout [128, 2, D]
    return np.ascontiguousarray(wT.reshape(2, 128, D).transpose(1, 0, 2)).astype(NP_DT)


def _qk_chunks(x):
    # [S, D] -> [CH, 128 part(d%128), KT(d//128), 512(s in chunk)]
    return np.ascontiguousarray(
        x.reshape(CH, 512, KT, 128).transpose(0, 3, 2, 1)
    ).astype(NP_DT)


def _v_blocks(x):
    # [S, D] -> [SB, 128 part(d%128), KT(d//128), 128(s in block)]
    return np.ascontiguousarray(
        x.reshape(SB, 128, KT, 128).transpose(0, 3, 2, 1)
    ).astype(NP_DT)


def make_in_maps(q, k, v, mask, Wq, bq, Wk, bk, Wv, bv, Wf, bf):
    scale = 1.0 / np.sqrt(np.float32(HD))
    f32 = np.float32
    m = np.asarray(mask[0, 0])
    # lhsT for the mask preload: lhsT.T @ I == maskT_add[k, q]
    tri = np.where(m[:128, :128] == 0, f32(-1e9), f32(0.0)).astype(NP_DT)
    idn = np.eye(128, dtype=NP_DT)
    sel = np.zeros((4, 4 * HD), np.float16)
    for r in range(4):
        sel[r, HD * r : HD * (r + 1)] = 1.0
    in_maps = []
    for c in range(8):
        b, g = c // 4, c % 4
        sl = slice(g * PL, (g + 1) * PL)
        in_maps.append(
            {
                "qc": _qk_chunks(np.asarray(q[b]).astype(f32) * scale),
                "kc": _qk_chunks(np.asarray(k[b])),
                "vc": _v_blocks(np.asarray(v[b])),
                "wq": _wlayout(np.asarray(Wq)[sl, :].T),
                "wk": _wlayout(np.asarray(Wk)[sl, :].T),
                "wv": _wlayout(np.asarray(Wv)[sl, :].T),
                "wf": _flayout(np.asarray(Wf)[:, sl].T),
                "bq2": np.ascontiguousarray((np.asarray(bq)[sl] * scale).astype(f32).reshape(2, 128).T),
                "bk2": np.ascontiguousarray(np.asarray(bk)[sl].astype(f32).reshape(2, 128).T),
                "bv1": np.asarray(bv)[sl].astype(f32).reshape(1, PL),
                "bf1": (np.asarray(bf).astype(f32) / 4.0).reshape(1, D),
                "tri": tri,
                "idn": idn,
                "sel": sel,
            }
        )
    return in_maps


def _mask_is_causal(mask):
    m = np.asarray(mask[0, 0])
    return bool(np.array_equal(m != 0, np.tril(np.ones((S, S), bool))))


def _numpy_fallback(q, k, v, mask, Wq, bq, Wk, bk, Wv, bv, Wf, bf):
    out = np.empty((B, S, D), np.float32)
    m = np.asarray(mask[0, 0])
    for b in range(B):
        qh = (np.asarray(q[b]) @ np.asarray(Wq).T + bq).reshape(S, H, HD)
        kh = (np.asarray(k[b]) @ np.asarray(Wk).T + bk).reshape(S, H, HD)
        vh = (np.asarray(v[b]) @ np.asarray(Wv).T + bv).reshape(S, H, HD)
        x = np.empty((S, H, HD), np.float32)
        for hh in range(H):
            sc = qh[:, hh] @ kh[:, hh].T / np.sqrt(np.float32(HD))
            sc = np.where(m == 0, np.float32(-1e9), sc)
            sc = sc - sc.max(-1, keepdims=True)
            e = np.exp(sc)
            x[:, hh] = (e / e.sum(-1, keepdims=True)) @ vh[:, hh]
        out[b] = x.reshape(S, D) @ np.asarray(Wf).T + bf
    return out


def kernel(q, k, v, mask, Wq, bq, Wk, bk, Wv, bv, Wf, bf):
    global last_results
    if not _mask_is_causal(mask):
        return _numpy_fallback(q, k, v, mask, Wq, bq, Wk, bk, Wv, bv, Wf, bf)
    nc = build_program()
    in_maps = make_in_maps(q, k, v, mask, Wq, bq, Wk, bk, Wv, bv, Wf, bf)
    res = run_bass_kernel_spmd(nc, in_maps, core_ids=list(range(8)))
    last_results = res
    out = np.zeros((B, S, D), np.float32)
    for c in range(8):
        out[c // 4] += res.results[c]["out"].astype(np.float32)
    return out


# revision 26
# speedup vs baseline: 1.0318x; 1.0318x over previous
"""Multi-head attention (B=2, S=2048, D=1024, H=16) on 8 TRN2 NeuronCores.

Sharding: batch x head-group. Core c handles batch b=c//4 and heads
[4g, 4g+4) with g=c%4 (column-parallel QKV projections, row-parallel
output projection). Each core emits a partial [S, D] output; the host
sums the 4 partials per batch (the row-parallel all-reduce).

Chunk-major pipeline (v2): the sequence is processed in 4 chunks of 512
queries. Per chunk: project q/k for that chunk (+v for its key blocks),
run both head-pairs' scores->exp->AV, normalize, and do the output
projection — so softmax normalization and the out-projection of chunk c
hide under the attention of chunk c+1 instead of forming a serial tail.
Host re-layouts q/k/v so every chunk's input is one contiguous DMA.

Other key choices (all matmuls bf16 with f32 PSUM accum):
- Scores are computed transposed (scoresT[k, q]); the K=64 head-pair
  matmuls run concurrently in the PE array via base-partition row
  tiling. vh carries a ones column so attention row-sums fall out of
  the AV matmul.
- Causal masking of diagonal blocks is a PE preload: a [128,128]
  additive -1e9 matmul with start=True, which the score matmul then
  accumulates onto (cols beyond 128 are overwritten since their
  has_written bits stay clear) — no VectorE masking pass.
- Softmax renormalization: row sums gathered to a [4,512] tile, 1/x on
  ScalarE as exp(-ln(x)) (both fns in one ACT table set), broadcast
  across partitions by a tiny one-hot fp16 matmul, applied by VectorE.
- ~80 junk warmup matmuls at t=0 keep the PE HAM clock-gate warm while
  the first input DMAs land.
"""

import os
import numpy as np
import ml_dtypes

import concourse.bass as bass
import concourse.tile as tile
from concourse import bacc, mybir
from concourse.bass_utils import run_bass_kernel_spmd

B, S, D, H = 2, 2048, 1024, 16
HD = D // H          # 64
HL = H // 4          # 4 heads per core
PL = HL * HD         # 256 local projection dim
KT = D // 128        # 8 contraction blocks
SB = S // 128        # 16 sequence blocks of 128
CH = S // 512        # 4 sequence chunks of 512
F32 = mybir.dt.float32
F16 = mybir.dt.float16
DT = mybir.dt.bfloat16
NP_DT = ml_dtypes.bfloat16
USE_ACT_RECIP = False  # Ln+Exp thrash ACT table sets (9 loads); DVE recip hides under the skew

_cache = {}
last_results = None


def build_program():
    if "nc" in _cache:
        return _cache["nc"]
    nc = bacc.Bacc("TRN2", target_bir_lowering=False, debug=False, num_devices=8)

    # inputs, host-relaid so every DMA is contiguous with >=2KB/partition
    qc_d = nc.dram_tensor("qc", [CH, 128, KT, 512], DT, kind="ExternalInput")
    kc_d = nc.dram_tensor("kc", [CH, 128, KT, 512], DT, kind="ExternalInput")
    vc_d = nc.dram_tensor("vc", [SB, 128, KT, 128], DT, kind="ExternalInput")
    wq_d = nc.dram_tensor("wq", [128, KT, PL], DT, kind="ExternalInput")
    wk_d = nc.dram_tensor("wk", [128, KT, PL], DT, kind="ExternalInput")
    wv_d = nc.dram_tensor("wv", [128, KT, PL], DT, kind="ExternalInput")
    wf_d = nc.dram_tensor("wf", [128, 2, D], DT, kind="ExternalInput")
    bq_d = nc.dram_tensor("bq2", [128, 2], F32, kind="ExternalInput")
    bk_d = nc.dram_tensor("bk2", [128, 2], F32, kind="ExternalInput")
    bv_d = nc.dram_tensor("bv1", [1, PL], F32, kind="ExternalInput")
    bf_d = nc.dram_tensor("bf1", [1, D], F32, kind="ExternalInput")
    tri_d = nc.dram_tensor("tri", [128, 128], DT, kind="ExternalInput")
    idn_d = nc.dram_tensor("idn", [128, 128], DT, kind="ExternalInput")
    sel_d = nc.dram_tensor("sel", [4, 4 * HD], F16, kind="ExternalInput")
    out_d = nc.dram_tensor("out", [S, D], DT, kind="ExternalOutput")

    ADD = mybir.AluOpType.add
    MUL = mybir.AluOpType.mult
    EXP = mybir.ActivationFunctionType.Exp
    LN = mybir.ActivationFunctionType.Ln

    with tile.TileContext(nc) as tc:
        with (
            tc.tile_pool(name="singles", bufs=1) as singles,
            tc.tile_pool(name="qk", bufs=3) as qkp,
            tc.tile_pool(name="vin", bufs=6) as vinp,
            tc.tile_pool(name="epool", bufs=6) as epool,
            tc.tile_pool(name="apool", bufs=6) as apool,
            tc.tile_pool(name="npool", bufs=2) as npool,
            tc.tile_pool(name="opool", bufs=2) as opool,
            tc.tile_pool(name="psum", bufs=2, space="PSUM") as psum,
        ):
            wq_sb = singles.tile([128, KT, PL], DT)
            wk_sb = singles.tile([128, KT, PL], DT)
            wv_sb = singles.tile([128, KT, PL], DT)
            wf_sb = singles.tile([128, 2, D], DT)
            bq_sb = singles.tile([128, 2], F32)
            bk_sb = singles.tile([128, 2], F32)
            bv_bc = singles.tile([128, PL], F32)
            bf_bc = singles.tile([128, D], F32)
            tri_sb = singles.tile([128, 128], DT)
            idn_sb = singles.tile([128, 128], DT)
            sel_sb = singles.tile([4, 4 * HD], F16)

            qhT = singles.tile([128, 2, S], DT)   # [p within pair, pair, s]
            khT = singles.tile([128, 2, S], DT)
            vh = singles.tile([128, SB, HL, HD + 1], DT)  # [s in blk, sblk, h, hd|1]
            xn = singles.tile([128, 2, S], DT)    # normalized attn out
            nc.vector.memset(vh[:, :, :, HD : HD + 1], 1.0)

            # junk matmuls keep the PE HAM activity window busy across
            # dependency stalls (initial DMA wait, hb boundaries, the
            # epilogue's reciprocal chain) so real matmuls run at 2.4 GHz
            wu = singles.tile([128, 128], DT)
            nc.vector.memset(wu, 0.0)

            def junk(n):
                for _ in range(n):
                    wp = psum.tile([128, 64], F32, tag="C", bufs=2, name="wu")
                    nc.tensor.matmul(
                        wp, lhsT=wu, rhs=wu[:, 0:64], start=True, stop=True
                    )

            junk(80)

            # weights first (small), then per-chunk inputs
            nc.sync.dma_start(wq_sb, wq_d.ap())
            nc.sync.dma_start(bq_sb, bq_d.ap())
            nc.sync.dma_start(wk_sb, wk_d.ap())
            nc.sync.dma_start(bk_sb, bk_d.ap())
            nc.sync.dma_start(wv_sb, wv_d.ap())
            nc.sync.dma_start(tri_sb, tri_d.ap())
            nc.sync.dma_start(idn_sb, idn_d.ap())
            nc.sync.dma_start(bv_bc, bv_d.ap().to_broadcast([128, PL]))

            # DMA issue is decoupled from compute: inputs for chunk c+1 are
            # DMA'd at the start of chunk c, and their projections are
            # emitted as PE "filler" between attention blocks of chunk c
            # (the PE queue is strictly in-order, so without filler it
            # would idle every block waiting on ScalarE's exp).
            def issue_qk(c):
                xq = qkp.tile([128, KT, 512], DT, tag="xin")
                nc.sync.dma_start(xq, qc_d.ap()[c])
                xk = qkp.tile([128, KT, 512], DT, tag="xin")
                nc.sync.dma_start(xk, kc_d.ap()[c])
                return xq, xk

            def issue_v(c):
                vts = []
                for sb in range(4 * c, 4 * c + 4):
                    vt = vinp.tile([128, KT, 128], DT, tag="vin", bufs=8)
                    nc.sync.dma_start(vt, vc_d.ap()[sb])
                    vts.append(vt)
                return vts

            def proj_item(xt, c, pt, w_sb, b_sb, out_sb):
                pp = psum.tile([128, 512], F32, tag="C", bufs=2, name="pp")
                for kk in range(KT):
                    nc.tensor.matmul(
                        pp,
                        lhsT=w_sb[:, kk, 128 * pt : 128 * (pt + 1)],
                        rhs=xt[:, kk, :],
                        start=(kk == 0),
                        stop=(kk == KT - 1),
                    )
                nc.vector.tensor_scalar_add(
                    out_sb[:, pt, 512 * c : 512 * (c + 1)],
                    pp,
                    b_sb[:, pt : pt + 1],
                )

            def vproj_item(vt, sb):
                pv = psum.tile([128, PL], F32, tag="C", bufs=2, name="pv")
                for kk in range(KT):
                    nc.tensor.matmul(
                        pv,
                        lhsT=vt[:, kk, :],
                        rhs=wv_sb[:, kk, :],
                        start=(kk == 0),
                        stop=(kk == KT - 1),
                    )
                nc.vector.tensor_tensor(
                    out=vh[:, sb, :, 0:HD],
                    in0=pv.rearrange("p (h e) -> p h e", h=HL),
                    in1=bv_bc.rearrange("p (h e) -> p h e", h=HL),
                    op=ADD,
                )

            def norm_item(c, r, xas, rec16, krows=4):
                # rec16 has `krows` partitions; row r % krows holds head r's
                # 1/rowsum. sel's one-hot layout makes any [krows, 64] slice
                # at column 64r a valid broadcast selector.
                hb, par = r // 2, r % 2
                rb = psum.tile([HD, 512], F32, tag="C", bufs=2, name="rb")
                nc.tensor.matmul(
                    rb,
                    lhsT=sel_sb[0:krows, HD * (r % krows) : HD * (r % krows + 1)],
                    rhs=rec16,
                    start=True,
                    stop=True,
                )
                xt_n = npool.tile([HD, 512], DT, tag="xtn", bufs=4)
                nc.vector.tensor_tensor(out=xt_n, in0=xas[r][0:HD, :], in1=rb, op=MUL)
                nc.sync.dma_start(
                    xn[64 * par : 64 * par + 64, hb, 512 * c : 512 * (c + 1)], xt_n
                )

            def d_item(c, ib, split_dma=False):
                ob = opool.tile([128, D], DT, tag="ob")
                for oc in range(2):
                    po = psum.tile([128, 512], F32, tag="C", bufs=2, name="po")
                    for t in range(2):
                        nc.tensor.matmul(
                            po,
                            lhsT=xn[:, t, 128 * ib : 128 * (ib + 1)],
                            rhs=wf_sb[:, t, 512 * oc : 512 * (oc + 1)],
                            start=(t == 0),
                            stop=(t == 1),
                        )
                    nc.vector.tensor_tensor(
                        out=ob[:, 512 * oc : 512 * (oc + 1)],
                        in0=po,
                        in1=bf_bc[:, 512 * oc : 512 * (oc + 1)],
                        op=ADD,
                    )
                    if split_dma:
                        nc.sync.dma_start(
                            out_d.ap()[
                                128 * ib : 128 * (ib + 1),
                                512 * oc : 512 * (oc + 1),
                            ],
                            ob[:, 512 * oc : 512 * (oc + 1)],
                        )
                if not split_dma:
                    nc.sync.dma_start(out_d.ap()[128 * ib : 128 * (ib + 1), :], ob)

            def attn_chunk(c, filler, split_norm=False):
                """Attention for chunk c, interleaving `filler` PE work into
                the exp-latency gaps. AV(bj) is emitted one block late so
                scores(bj+1)+filler cover exp(bj)'s ScalarE latency.
                split_norm (last chunk): reciprocal runs per head-pair so
                hb0's normalize joins the filler during hb1's attention."""
                xas = {}
                recs = {}
                sums = npool.tile([4, 512], F32, tag="sums", bufs=3)
                nbj = 4 * c + 4
                slots = 2 * nbj

                def pop_filler(n=None):
                    # floor pacing leaves a remainder that flushes at the
                    # hb/chunk boundaries — exactly where the PE would
                    # otherwise idle on psp-slot waits and HAM rethrottles
                    nonlocal slots
                    if n is None:
                        n = len(filler) // slots if slots > 0 else 0
                    for _ in range(min(n, len(filler))):
                        filler.pop(0)()
                    slots = max(slots - 1, 0)

                for hb in range(2):
                    pxs = [
                        psum.tile([128, 512], F32, tag="B", bufs=2, name=f"px{p_}")
                        for p_ in range(2)
                    ]
                    pend_av = None
                    for bj in range(nbj):
                        band = bj >= 4 * c
                        i0 = 128 * bj if band else 512 * c
                        w = 512 * (c + 1) - i0
                        o = i0 - 512 * c
                        psp = psum.tile([128, 2, 512], F32, tag="A", bufs=2)
                        for par in range(2):
                            nc.tensor.matmul(
                                psp[:, par, 0:w],
                                lhsT=khT[
                                    64 * par : 64 * par + 64,
                                    hb,
                                    128 * bj : 128 * (bj + 1),
                                ],
                                rhs=qhT[64 * par : 64 * par + 64, hb, i0 : i0 + w],
                                start=True,
                                stop=not band,
                            )
                        if band:
                            # causal masking on the PE: accumulate a -1e9
                            # upper-triangle matmul onto the diagonal
                            # 128x128 sub-block (cols 0:128)
                            for par in range(2):
                                nc.tensor.matmul(
                                    psp[:, par, 0:128],
                                    lhsT=tri_sb,
                                    rhs=idn_sb,
                                    start=False,
                                    stop=True,
                                )
                        et = epool.tile([128, 2, 512], DT, tag="et")
                        nc.scalar.activation(et[:, :, 0:w], psp[:, :, 0:w], EXP)
                        if pend_av is not None:
                            pop_filler()
                            pend_av()
                        pend_av = (
                            lambda et=et, w=w, o=o, bj=bj: [
                                nc.tensor.matmul(
                                    pxs[par][0 : HD + 1, o : o + w],
                                    lhsT=vh[:, bj, 2 * hb + par, :],
                                    rhs=et[:, par, 0:w],
                                    start=(bj == 0),
                                    stop=(bj == nbj - 1),
                                )
                                for par in range(2)
                            ]
                        )
                    pop_filler()
                    junk(6)  # cover the last exp's lag at the hb boundary
                    pend_av()
                    pop_filler(2)  # boundary flush: cover the psp-slot wait
                    junk(4)
                    if split_norm:
                        sums_h = npool.tile([2, 512], F32, tag="sums2", name="sums_h")
                    else:
                        sums_h = sums
                    for par in range(2):
                        r = 2 * hb + par
                        xa = apool.tile([HD + 1, 512], F32, tag="xa", bufs=10)
                        nc.vector.tensor_copy(out=xa, in_=pxs[par][0 : HD + 1, :])
                        row = par if split_norm else r
                        nc.sync.dma_start(sums_h[row : row + 1, :], xa[HD : HD + 1, :])
                        xas[r] = xa
                    if split_norm:
                        if hb == 0:
                            rec = npool.tile([2, 512], F32, tag="rec2")
                            nc.vector.reciprocal(rec, sums_h)
                            r16 = npool.tile([2, 512], F16, tag="rec16b")
                            nc.vector.tensor_copy(r16, rec)
                            for r in range(2):
                                filler.append(
                                    lambda r=r, r16=r16: norm_item(
                                        c, r, xas, r16, krows=2
                                    )
                                )
                        else:
                            recs[1] = sums_h  # epilogue pipelines the recip
                if split_norm:
                    # drain leftover filler (hb0's normalize rides here)
                    for f in filler:
                        f()
                    return xas, recs[1]
                rec16 = npool.tile([4, 512], F16, tag="rec16")
                rec = npool.tile([4, 512], F32, tag="rec")
                nc.vector.reciprocal(rec, sums)
                nc.vector.tensor_copy(rec16, rec)
                # drain any leftover filler
                for f in filler:
                    f()
                return xas, rec16

            # ---- prologue: chunk 0 inputs + projections, serial ----
            xq0, xk0 = issue_qk(0)
            vts0 = issue_v(0)
            for pt in range(2):
                proj_item(xq0, 0, pt, wq_sb, bq_sb, qhT)
            for pt in range(2):
                proj_item(xk0, 0, pt, wk_sb, bk_sb, khT)
            for j, sb in enumerate(range(0, 4)):
                vproj_item(vts0[j], sb)

            pending = None
            for c in range(CH):
                filler = []
                if c + 1 < CH:
                    xq, xk = issue_qk(c + 1)
                    vts = issue_v(c + 1)
                    if c == 0:
                        nc.sync.dma_start(wf_sb, wf_d.ap())
                        nc.sync.dma_start(bf_bc, bf_d.ap().to_broadcast([128, D]))
                        nc.sync.dma_start(sel_sb, sel_d.ap())
                if pending is not None:
                    pc, pxas, prec16 = pending
                    for r in range(4):
                        filler.append(
                            lambda r=r: norm_item(pc, r, pxas, prec16)
                        )
                if c + 1 < CH:
                    for pt in range(2):
                        filler.append(
                            lambda pt=pt, xq=xq, cn=c + 1: proj_item(
                                xq, cn, pt, wq_sb, bq_sb, qhT
                            )
                        )
                    for pt in range(2):
                        filler.append(
                            lambda pt=pt, xk=xk, cn=c + 1: proj_item(
                                xk, cn, pt, wk_sb, bk_sb, khT
                            )
                        )
                if pending is not None:
                    pc = pending[0]
                    for ib in range(4 * pc, 4 * pc + 4):
                        filler.append(lambda ib=ib, pc=pc: d_item(pc, ib))
                if c + 1 < CH:
                    for j, sb in enumerate(range(4 * (c + 1), 4 * (c + 1) + 4)):
                        filler.append(
                            lambda j=j, sb=sb, vts=vts: vproj_item(vts[j], sb)
                        )
                state = attn_chunk(c, filler, split_norm=(c == CH - 1))
                pending = (c, *state)

            # ---- epilogue: last chunk's pair-1 normalize + out-projection.
            # (pair 0 was normalized as filler during the last chunk's hb1.)
            # Junk matmuls sandwich the normalize so the PE stays HAM-warm
            # through the reciprocal and the xn-placement DMA latencies.
            junk(56)
            pc, pxas, sums_h1 = pending
            rec = npool.tile([2, 512], F32, tag="rec2", name="rec_e")
            nc.vector.reciprocal(rec, sums_h1)
            r16 = npool.tile([2, 512], F16, tag="rec16b", name="r16_e")
            nc.vector.tensor_copy(r16, rec)
            for r in (2, 3):
                norm_item(pc, r, pxas, r16, krows=2)
            junk(30)
            for ib in range(4 * pc, 4 * pc + 4):
                d_item(pc, ib, split_dma=(ib == 4 * pc + 3))

    nc.compile()
    _cache["nc"] = nc
    return nc


def _wlayout(wT):
    # [D, PL] -> SBUF layout [128, KT, PL]
    return np.ascontiguousarray(wT.reshape(KT, 128, PL).transpose(1, 0, 2)).astype(NP_DT)


def _flayout(wT):
    # [PL, D] -> SBUF layout [128, 2, D]
    return np.ascontiguousarray(wT.reshape(2, 128, D).transpose(1, 0, 2)).astype(NP_DT)


def _qk_chunks(x):
    # [S, D] -> [CH, 128 part(d%128), KT(d//128), 512(s in chunk)]
    return np.ascontiguousarray(
        x.reshape(CH, 512, KT, 128).transpose(0, 3, 2, 1)
    ).astype(NP_DT)


def _v_blocks(x):
    # [S, D] -> [SB, 128 part(d%128), KT(d//128), 128(s in block)]
    return np.ascontiguousarray(
        x.reshape(SB, 128, KT, 128).transpose(0, 3, 2, 1)
    ).astype(NP_DT)


def make_in_maps(q, k, v, mask, Wq, bq, Wk, bk, Wv, bv, Wf, bf):
    scale = 1.0 / np.sqrt(np.float32(HD))
    f32 = np.float32
    m = np.asarray(mask[0, 0])
    # lhsT for the mask preload: lhsT.T @ I == maskT_add[k, q]
    tri = np.where(m[:128, :128] == 0, f32(-1e9), f32(0.0)).astype(NP_DT)
    idn = np.eye(128, dtype=NP_DT)
    sel = np.zeros((4, 4 * HD), np.float16)
    for r in range(4):
        sel[r, HD * r : HD * (r + 1)] = 1.0
    in_maps = []
    for c in range(8):
        b, g = c // 4, c % 4
        sl = slice(g * PL, (g + 1) * PL)
        in_maps.append(
            {
                "qc": _qk_chunks(np.asarray(q[b]).astype(f32) * scale),
                "kc": _qk_chunks(np.asarray(k[b])),
                "vc": _v_blocks(np.asarray(v[b])),
                "wq": _wlayout(np.asarray(Wq)[sl, :].T),
                "wk": _wlayout(np.asarray(Wk)[sl, :].T),
                "wv": _wlayout(np.asarray(Wv)[sl, :].T),
                "wf": _flayout(np.asarray(Wf)[:, sl].T),
                "bq2": np.ascontiguousarray((np.asarray(bq)[sl] * scale).astype(f32).reshape(2, 128).T),
                "bk2": np.ascontiguousarray(np.asarray(bk)[sl].astype(f32).reshape(2, 128).T),
                "bv1": np.asarray(bv)[sl].astype(f32).reshape(1, PL),
                "bf1": (np.asarray(bf).astype(f32) / 4.0).reshape(1, D),
                "tri": tri,
                "idn": idn,
                "sel": sel,
            }
        )
    return in_maps


def _mask_is_causal(mask):
    m = np.asarray(mask[0, 0])
    return bool(np.array_equal(m != 0, np.tril(np.ones((S, S), bool))))


def _numpy_fallback(q, k, v, mask, Wq, bq, Wk, bk, Wv, bv, Wf, bf):
    out = np.empty((B, S, D), np.float32)
    m = np.asarray(mask[0, 0])
    for b in range(B):
        qh = (np.asarray(q[b]) @ np.asarray(Wq).T + bq).reshape(S, H, HD)
        kh = (np.asarray(k[b]) @ np.asarray(Wk).T + bk).reshape(S, H, HD)
        vh = (np.asarray(v[b]) @ np.asarray(Wv).T + bv).reshape(S, H, HD)
        x = np.empty((S, H, HD), np.float32)
        for hh in range(H):
            sc = qh[:, hh] @ kh[:, hh].T / np.sqrt(np.float32(HD))
            sc = np.where(m == 0, np.float32(-1e9), sc)
            sc = sc - sc.max(-1, keepdims=True)
            e = np.exp(sc)
            x[:, hh] = (e / e.sum(-1, keepdims=True)) @ vh[:, hh]
        out[b] = x.reshape(S, D) @ np.asarray(Wf).T + bf
    return out


def kernel(q, k, v, mask, Wq, bq, Wk, bk, Wv, bv, Wf, bf):
    global last_results
    if not _mask_is_causal(mask):
        return _numpy_fallback(q, k, v, mask, Wq, bq, Wk, bk, Wv, bv, Wf, bf)
    nc = build_program()
    in_maps = make_in_maps(q, k, v, mask, Wq, bq, Wk, bk, Wv, bv, Wf, bf)
    res = run_bass_kernel_spmd(nc, in_maps, core_ids=list(range(8)))
    last_results = res
    out = np.zeros((B, S, D), np.float32)
    for c in range(8):
        out[c // 4] += res.results[c]["out"].astype(np.float32)
    return out


# revision 31
# speedup vs baseline: 1.0897x; 1.0562x over previous
"""Multi-head attention (B=2, S=2048, D=1024, H=16) on 8 TRN2 NeuronCores.

Sharding: batch x head-group. Core c handles batch b=c//4 and heads
[4g, 4g+4) with g=c%4 (column-parallel QKV projections, row-parallel
output projection). Each core emits a partial [S, D] output; the host
sums the 4 partials per batch (the row-parallel all-reduce).

Chunk-major pipeline (v2): the sequence is processed in 4 chunks of 512
queries. Per chunk: project q/k for that chunk (+v for its key blocks),
run both head-pairs' scores->exp->AV, normalize, and do the output
projection — so softmax normalization and the out-projection of chunk c
hide under the attention of chunk c+1 instead of forming a serial tail.
Host re-layouts q/k/v so every chunk's input is one contiguous DMA.

Other key choices (all matmuls bf16 with f32 PSUM accum):
- Scores are computed transposed (scoresT[k, q]); the K=64 head-pair
  matmuls run concurrently in the PE array via base-partition row
  tiling. vh carries a ones column so attention row-sums fall out of
  the AV matmul.
- Causal masking of diagonal blocks is a PE preload: a [128,128]
  additive -1e9 matmul with start=True, which the score matmul then
  accumulates onto (cols beyond 128 are overwritten since their
  has_written bits stay clear) — no VectorE masking pass.
- Softmax renormalization: row sums gathered to a [4,512] tile, 1/x on
  ScalarE as exp(-ln(x)) (both fns in one ACT table set), broadcast
  across partitions by a tiny one-hot fp16 matmul, applied by VectorE.
- ~80 junk warmup matmuls at t=0 keep the PE HAM clock-gate warm while
  the first input DMAs land.
"""

import os
import numpy as np
import ml_dtypes

import concourse.bass as bass
import concourse.tile as tile
from concourse import bacc, mybir
from concourse.bass_utils import run_bass_kernel_spmd

B, S, D, H = 2, 2048, 1024, 16
HD = D // H          # 64
HL = H // 4          # 4 heads per core
PL = HL * HD         # 256 local projection dim
KT = D // 128        # 8 contraction blocks
SB = S // 128        # 16 sequence blocks of 128
CH = S // 512        # 4 sequence chunks of 512
F32 = mybir.dt.float32
F16 = mybir.dt.float16
DT = mybir.dt.bfloat16
NP_DT = ml_dtypes.bfloat16
USE_ACT_RECIP = False  # Ln+Exp thrash ACT table sets (9 loads); DVE recip hides under the skew

_cache = {}
last_results = None


def build_program():
    if "nc" in _cache:
        return _cache["nc"]
    nc = bacc.Bacc("TRN2", target_bir_lowering=False, debug=False, num_devices=8)

    # inputs, host-relaid so every DMA is contiguous with >=2KB/partition
    qc_d = nc.dram_tensor("qc", [CH, 128, KT, 512], DT, kind="ExternalInput")
    kc_d = nc.dram_tensor("kc", [CH, 128, KT, 512], DT, kind="ExternalInput")
    vc_d = nc.dram_tensor("vc", [SB, 128, KT, 128], DT, kind="ExternalInput")
    wq_d = nc.dram_tensor("wq", [128, KT, PL], DT, kind="ExternalInput")
    wk_d = nc.dram_tensor("wk", [128, KT, PL], DT, kind="ExternalInput")
    wv_d = nc.dram_tensor("wv", [128, KT, PL], DT, kind="ExternalInput")
    wf_d = nc.dram_tensor("wf", [128, 2, D], DT, kind="ExternalInput")
    bq_d = nc.dram_tensor("bq2", [128, 2], F32, kind="ExternalInput")
    bk_d = nc.dram_tensor("bk2", [128, 2], F32, kind="ExternalInput")
    bv_d = nc.dram_tensor("bv1", [1, PL], F32, kind="ExternalInput")
    bf_d = nc.dram_tensor("bf1", [1, D], F32, kind="ExternalInput")
    tri_d = nc.dram_tensor("tri", [128, 128], DT, kind="ExternalInput")
    idn_d = nc.dram_tensor("idn", [128, 128], DT, kind="ExternalInput")
    sel_d = nc.dram_tensor("sel", [4, 4 * HD], F16, kind="ExternalInput")
    out_d = nc.dram_tensor("out", [S, D], DT, kind="ExternalOutput")

    ADD = mybir.AluOpType.add
    MUL = mybir.AluOpType.mult
    EXP = mybir.ActivationFunctionType.Exp
    LN = mybir.ActivationFunctionType.Ln

    with tile.TileContext(nc) as tc:
        with (
            tc.tile_pool(name="singles", bufs=1) as singles,
            tc.tile_pool(name="qk", bufs=3) as qkp,
            tc.tile_pool(name="vin", bufs=6) as vinp,
            tc.tile_pool(name="epool", bufs=6) as epool,
            tc.tile_pool(name="apool", bufs=6) as apool,
            tc.tile_pool(name="npool", bufs=2) as npool,
            tc.tile_pool(name="opool", bufs=2) as opool,
            tc.tile_pool(name="psum", bufs=2, space="PSUM") as psum,
        ):
            wq_sb = singles.tile([128, KT, PL], DT)
            wk_sb = singles.tile([128, KT, PL], DT)
            wv_sb = singles.tile([128, KT, PL], DT)
            wf_sb = singles.tile([128, 2, D], DT)
            bq_sb = singles.tile([128, 2], F32)
            bk_sb = singles.tile([128, 2], F32)
            bv_bc = singles.tile([128, PL], F32)
            bf_bc = singles.tile([128, D], F32)
            tri_sb = singles.tile([128, 128], DT)
            idn_sb = singles.tile([128, 128], DT)
            sel_sb = singles.tile([4, 4 * HD], F16)

            qhT = singles.tile([128, 2, S], DT)   # [p within pair, pair, s]
            khT = singles.tile([128, 2, S], DT)
            vh = singles.tile([128, SB, HL, HD + 1], DT)  # [s in blk, sblk, h, hd|1]
            xn = singles.tile([128, 2, S], DT)    # normalized attn out
            nc.vector.memset(vh[:, :, :, HD : HD + 1], 1.0)

            # junk matmuls keep the PE HAM activity window busy across
            # dependency stalls (initial DMA wait, hb boundaries, the
            # epilogue's reciprocal chain) so real matmuls run at 2.4 GHz
            wu = singles.tile([128, 128], DT)
            nc.vector.memset(wu, 0.0)

            def junk(n, w=64):
                for _ in range(n):
                    wp = psum.tile([128, 128], F32, tag="C", bufs=2, name="wu")
                    nc.tensor.matmul(
                        wp[:, 0:w], lhsT=wu, rhs=wu[:, 0:w], start=True, stop=True
                    )

            junk(64)

            # DMA issue is decoupled from compute: inputs for chunk c+1 are
            # DMA'd at the start of chunk c, and their projections are
            # emitted as PE "filler" between attention blocks of chunk c
            # (the PE queue is strictly in-order, so without filler it
            # would idle every block waiting on ScalarE's exp).
            def issue_x(x_d, c):
                xt = qkp.tile([128, KT, 512], DT, tag="xin", name="xt")
                nc.sync.dma_start(xt, x_d.ap()[c])
                return xt

            def issue_qk(c):
                return issue_x(qc_d, c), issue_x(kc_d, c)

            def issue_v(c):
                vts = []
                for sb in range(4 * c, 4 * c + 4):
                    vt = vinp.tile([128, KT, 128], DT, tag="vin", bufs=8)
                    nc.sync.dma_start(vt, vc_d.ap()[sb])
                    vts.append(vt)
                return vts

            def proj_item(xt, c, pt, w_sb, b_sb, out_sb):
                pp = psum.tile([128, 512], F32, tag="C", bufs=2, name="pp")
                for kk in range(KT):
                    nc.tensor.matmul(
                        pp,
                        lhsT=w_sb[:, kk, 128 * pt : 128 * (pt + 1)],
                        rhs=xt[:, kk, :],
                        start=(kk == 0),
                        stop=(kk == KT - 1),
                    )
                nc.vector.tensor_scalar_add(
                    out_sb[:, pt, 512 * c : 512 * (c + 1)],
                    pp,
                    b_sb[:, pt : pt + 1],
                )

            def vproj_item(vt, sb):
                pv = psum.tile([128, PL], F32, tag="C", bufs=2, name="pv")
                for kk in range(KT):
                    nc.tensor.matmul(
                        pv,
                        lhsT=vt[:, kk, :],
                        rhs=wv_sb[:, kk, :],
                        start=(kk == 0),
                        stop=(kk == KT - 1),
                    )
                nc.vector.tensor_tensor(
                    out=vh[:, sb, :, 0:HD],
                    in0=pv.rearrange("p (h e) -> p h e", h=HL),
                    in1=bv_bc.rearrange("p (h e) -> p h e", h=HL),
                    op=ADD,
                )

            def norm_item(c, r, xas, rec16, krows=4):
                # rec16 has `krows` partitions; row r % krows holds head r's
                # 1/rowsum. sel's one-hot layout makes any [krows, 64] slice
                # at column 64r a valid broadcast selector.
                hb, par = r // 2, r % 2
                rb = psum.tile([HD, 512], F32, tag="C", bufs=2, name="rb")
                nc.tensor.matmul(
                    rb,
                    lhsT=sel_sb[0:krows, HD * (r % krows) : HD * (r % krows + 1)],
                    rhs=rec16,
                    start=True,
                    stop=True,
                )
                xt_n = npool.tile([HD, 512], DT, tag="xtn", bufs=4)
                nc.vector.tensor_tensor(out=xt_n, in0=xas[r][0:HD, :], in1=rb, op=MUL)
                nc.sync.dma_start(
                    xn[64 * par : 64 * par + 64, hb, 512 * c : 512 * (c + 1)], xt_n
                )

            def d_item(c, ib, split_dma=False):
                ob = opool.tile([128, D], DT, tag="ob")
                for oc in range(2):
                    po = psum.tile([128, 512], F32, tag="C", bufs=2, name="po")
                    for t in range(2):
                        nc.tensor.matmul(
                            po,
                            lhsT=xn[:, t, 128 * ib : 128 * (ib + 1)],
                            rhs=wf_sb[:, t, 512 * oc : 512 * (oc + 1)],
                            start=(t == 0),
                            stop=(t == 1),
                        )
                    nc.vector.tensor_tensor(
                        out=ob[:, 512 * oc : 512 * (oc + 1)],
                        in0=po,
                        in1=bf_bc[:, 512 * oc : 512 * (oc + 1)],
                        op=ADD,
                    )
                    if split_dma:
                        nc.sync.dma_start(
                            out_d.ap()[
                                128 * ib : 128 * (ib + 1),
                                512 * oc : 512 * (oc + 1),
                            ],
                            ob[:, 512 * oc : 512 * (oc + 1)],
                        )
                if not split_dma:
                    nc.sync.dma_start(out_d.ap()[128 * ib : 128 * (ib + 1), :], ob)

            def attn_chunk(c, filler, split_norm=False):
                """Attention for chunk c, interleaving `filler` PE work into
                the exp-latency gaps. AV(bj) is emitted one block late so
                scores(bj+1)+filler cover exp(bj)'s ScalarE latency.
                split_norm (last chunk): reciprocal runs per head-pair so
                hb0's normalize joins the filler during hb1's attention."""
                xas = {}
                recs = {}
                sums = npool.tile([4, 512], F32, tag="sums", bufs=3)
                nbj = 4 * c + 4
                slots = 2 * nbj

                def pop_filler(n=None):
                    # floor pacing leaves a remainder that flushes at the
                    # hb/chunk boundaries — exactly where the PE would
                    # otherwise idle on psp-slot waits and HAM rethrottles
                    nonlocal slots
                    if n is None:
                        n = len(filler) // slots if slots > 0 else 0
                    for _ in range(min(n, len(filler))):
                        filler.pop(0)()
                    slots = max(slots - 1, 0)

                for hb in range(2):
                    pxs = [
                        psum.tile([128, 512], F32, tag="B", bufs=2, name=f"px{p_}")
                        for p_ in range(2)
                    ]
                    pend_av = None
                    for bj in range(nbj):
                        band = bj >= 4 * c
                        i0 = 128 * bj if band else 512 * c
                        w = 512 * (c + 1) - i0
                        o = i0 - 512 * c
                        psp = psum.tile([128, 2, 512], F32, tag="A", bufs=2)
                        for par in range(2):
                            nc.tensor.matmul(
                                psp[:, par, 0:w],
                                lhsT=khT[
                                    64 * par : 64 * par + 64,
                                    hb,
                                    128 * bj : 128 * (bj + 1),
                                ],
                                rhs=qhT[64 * par : 64 * par + 64, hb, i0 : i0 + w],
                                start=True,
                                stop=not band,
                            )
                        if band:
                            # causal masking on the PE: accumulate a -1e9
                            # upper-triangle matmul onto the diagonal
                            # 128x128 sub-block (cols 0:128)
                            for par in range(2):
                                nc.tensor.matmul(
                                    psp[:, par, 0:128],
                                    lhsT=tri_sb,
                                    rhs=idn_sb,
                                    start=False,
                                    stop=True,
                                )
                        et = epool.tile([128, 2, 512], DT, tag="et")
                        nc.scalar.activation(et[:, :, 0:w], psp[:, :, 0:w], EXP)
                        if pend_av is not None:
                            pop_filler()
                            pend_av()
                        pend_av = (
                            lambda et=et, w=w, o=o, bj=bj: [
                                nc.tensor.matmul(
                                    pxs[par][0 : HD + 1, o : o + w],
                                    lhsT=vh[:, bj, 2 * hb + par, :],
                                    rhs=et[:, par, 0:w],
                                    start=(bj == 0),
                                    stop=(bj == nbj - 1),
                                )
                                for par in range(2)
                            ]
                        )
                    pop_filler()
                    junk(4)  # cover the last exp's lag at the hb boundary
                    pend_av()
                    pop_filler(2)  # boundary flush: cover the psp-slot wait
                    junk(2)
                    if split_norm:
                        sums_h = npool.tile([2, 512], F32, tag="sums2", name="sums_h")
                    else:
                        sums_h = sums
                    for par in range(2):
                        r = 2 * hb + par
                        xa = apool.tile([HD + 1, 512], F32, tag="xa", bufs=10)
                        nc.vector.tensor_copy(out=xa, in_=pxs[par][0 : HD + 1, :])
                        row = par if split_norm else r
                        nc.sync.dma_start(sums_h[row : row + 1, :], xa[HD : HD + 1, :])
                        xas[r] = xa
                    if split_norm:
                        if hb == 0:
                            rec = npool.tile([2, 512], F32, tag="rec2")
                            nc.vector.reciprocal(rec, sums_h)
                            r16 = npool.tile([2, 512], F16, tag="rec16b")
                            nc.vector.tensor_copy(r16, rec)
                            for r in range(2):
                                filler.append(
                                    lambda r=r, r16=r16: norm_item(
                                        c, r, xas, r16, krows=2
                                    )
                                )
                        else:
                            recs[1] = sums_h  # epilogue pipelines the recip
                if split_norm:
                    # drain leftover filler (hb0's normalize rides here)
                    for f in filler:
                        f()
                    return xas, recs[1]
                rec16 = npool.tile([4, 512], F16, tag="rec16")
                rec = npool.tile([4, 512], F32, tag="rec")
                nc.vector.reciprocal(rec, sums)
                nc.vector.tensor_copy(rec16, rec)
                # drain any leftover filler
                for f in filler:
                    f()
                return xas, rec16

            # ---- prologue: chunk 0, DMAs ordered so each projection's
            # inputs arrive just before the PE reaches it ----
            nc.sync.dma_start(wq_sb, wq_d.ap())
            nc.sync.dma_start(bq_sb, bq_d.ap())
            xq0 = issue_x(qc_d, 0)
            nc.sync.dma_start(wk_sb, wk_d.ap())
            nc.sync.dma_start(bk_sb, bk_d.ap())
            xk0 = issue_x(kc_d, 0)
            nc.sync.dma_start(wv_sb, wv_d.ap())
            nc.sync.dma_start(bv_bc, bv_d.ap().to_broadcast([128, PL]))
            vts0 = issue_v(0)
            nc.sync.dma_start(tri_sb, tri_d.ap())
            nc.sync.dma_start(idn_sb, idn_d.ap())
            for pt in range(2):
                proj_item(xq0, 0, pt, wq_sb, bq_sb, qhT)
            for pt in range(2):
                proj_item(xk0, 0, pt, wk_sb, bk_sb, khT)
            for j, sb in enumerate(range(0, 4)):
                vproj_item(vts0[j], sb)

            pending = None
            for c in range(CH):
                filler = []
                if c + 1 < CH:
                    xq, xk = issue_qk(c + 1)
                    vts = issue_v(c + 1)
                    if c == 0:
                        nc.sync.dma_start(wf_sb, wf_d.ap())
                        nc.sync.dma_start(bf_bc, bf_d.ap().to_broadcast([128, D]))
                        nc.sync.dma_start(sel_sb, sel_d.ap())
                if pending is not None:
                    pc, pxas, prec16 = pending
                    for r in range(4):
                        filler.append(
                            lambda r=r: norm_item(pc, r, pxas, prec16)
                        )
                if c + 1 < CH:
                    for pt in range(2):
                        filler.append(
                            lambda pt=pt, xq=xq, cn=c + 1: proj_item(
                                xq, cn, pt, wq_sb, bq_sb, qhT
                            )
                        )
                    for pt in range(2):
                        filler.append(
                            lambda pt=pt, xk=xk, cn=c + 1: proj_item(
                                xk, cn, pt, wk_sb, bk_sb, khT
                            )
                        )
                if pending is not None:
                    pc = pending[0]
                    for ib in range(4 * pc, 4 * pc + 4):
                        filler.append(lambda ib=ib, pc=pc: d_item(pc, ib))
                if c + 1 < CH:
                    for j, sb in enumerate(range(4 * (c + 1), 4 * (c + 1) + 4)):
                        filler.append(
                            lambda j=j, sb=sb, vts=vts: vproj_item(vts[j], sb)
                        )
                state = attn_chunk(c, filler, split_norm=(c == CH - 1))
                pending = (c, *state)

            # ---- epilogue: last chunk's pair-1 normalize + out-projection.
            # (pair 0 was normalized as filler during the last chunk's hb1.)
            # Junk matmuls sandwich the normalize so the PE stays HAM-warm
            # through the reciprocal and the xn-placement DMA latencies.
            junk(36, w=128)
            pc, pxas, sums_h1 = pending
            rec = npool.tile([2, 512], F32, tag="rec2", name="rec_e")
            nc.vector.reciprocal(rec, sums_h1)
            r16 = npool.tile([2, 512], F16, tag="rec16b", name="r16_e")
            nc.vector.tensor_copy(r16, rec)
            for r in (2, 3):
                norm_item(pc, r, pxas, r16, krows=2)
            junk(14, w=128)
            for ib in range(4 * pc, 4 * pc + 4):
                d_item(pc, ib, split_dma=(ib == 4 * pc + 3))

    nc.compile()
    _cache["nc"] = nc
    return nc


def _wlayout(wT):
    # [D, PL] -> SBUF layout [128, KT, PL]
    return np.ascontiguousarray(wT.reshape(KT, 128, PL).transpose(1, 0, 2)).astype(NP_DT)


def _flayout(wT):
    # [PL, D] -> SBUF layout [128, 2, D]
    return np.ascontiguousarray(wT.reshape(2, 128, D).transpose(1, 0, 2)).astype(NP_DT)


def _qk_chunks(x):
    # [S, D] -> [CH, 128 part(d%128), KT(d//128), 512(s in chunk)]
    return np.ascontiguousarray(
        x.reshape(CH, 512, KT, 128).transpose(0, 3, 2, 1)
    ).astype(NP_DT)


def _v_blocks(x):
    # [S, D] -> [SB, 128 part(d%128), KT(d//128), 128(s in block)]
    return np.ascontiguousarray(
        x.reshape(SB, 128, KT, 128).transpose(0, 3, 2, 1)
    ).astype(NP_DT)


def make_in_maps(q, k, v, mask, Wq, bq, Wk, bk, Wv, bv, Wf, bf):
    scale = 1.0 / np.sqrt(np.float32(HD))
    f32 = np.float32
    m = np.asarray(mask[0, 0])
    # lhsT for the mask preload: lhsT.T @ I == maskT_add[k, q]
    tri = np.where(m[:128, :128] == 0, f32(-1e9), f32(0.0)).astype(NP_DT)
    idn = np.eye(128, dtype=NP_DT)
    sel = np.zeros((4, 4 * HD), np.float16)
    for r in range(4):
        sel[r, HD * r : HD * (r + 1)] = 1.0
    in_maps = []
    for c in range(8):
        b, g = c // 4, c % 4
        sl = slice(g * PL, (g + 1) * PL)
        in_maps.append(
            {
                "qc": _qk_chunks(np.asarray(q[b]).astype(f32) * scale),
                "kc": _qk_chunks(np.asarray(k[b])),
                "vc": _v_blocks(np.asarray(v[b])),
                "wq": _wlayout(np.asarray(Wq)[sl, :].T),
                "wk": _wlayout(np.asarray(Wk)[sl, :].T),
                "wv": _wlayout(np.asarray(Wv)[sl, :].T),
                "wf": _flayout(np.asarray(Wf)[:, sl].T),
                "bq2": np.ascontiguousarray((np.asarray(bq)[sl] * scale).astype(f32).reshape(2, 128).T),
                "bk2": np.ascontiguousarray(np.asarray(bk)[sl].astype(f32).reshape(2, 128).T),
                "bv1": np.asarray(bv)[sl].astype(f32).reshape(1, PL),
                "bf1": (np.asarray(bf).astype(f32) / 4.0).reshape(1, D),
                "tri": tri,
                "idn": idn,
                "sel": sel,
            }
        )
    return in_maps


def _mask_is_causal(mask):
    m = np.asarray(mask[0, 0])
    return bool(np.array_equal(m != 0, np.tril(np.ones((S, S), bool))))


def _numpy_fallback(q, k, v, mask, Wq, bq, Wk, bk, Wv, bv, Wf, bf):
    out = np.empty((B, S, D), np.float32)
    m = np.asarray(mask[0, 0])
    for b in range(B):
        qh = (np.asarray(q[b]) @ np.asarray(Wq).T + bq).reshape(S, H, HD)
        kh = (np.asarray(k[b]) @ np.asarray(Wk).T + bk).reshape(S, H, HD)
        vh = (np.asarray(v[b]) @ np.asarray(Wv).T + bv).reshape(S, H, HD)
        x = np.empty((S, H, HD), np.float32)
        for hh in range(H):
            sc = qh[:, hh] @ kh[:, hh].T / np.sqrt(np.float32(HD))
            sc = np.where(m == 0, np.float32(-1e9), sc)
            sc = sc - sc.max(-1, keepdims=True)
            e = np.exp(sc)
            x[:, hh] = (e / e.sum(-1, keepdims=True)) @ vh[:, hh]
        out[b] = x.reshape(S, D) @ np.asarray(Wf).T + bf
    return out


def kernel(q, k, v, mask, Wq, bq, Wk, bk, Wv, bv, Wf, bf):
    global last_results
    if not _mask_is_causal(mask):
        return _numpy_fallback(q, k, v, mask, Wq, bq, Wk, bk, Wv, bv, Wf, bf)
    nc = build_program()
    in_maps = make_in_maps(q, k, v, mask, Wq, bq, Wk, bk, Wv, bv, Wf, bf)
    res = run_bass_kernel_spmd(nc, in_maps, core_ids=list(range(8)))
    last_results = res
    out = np.zeros((B, S, D), np.float32)
    for c in range(8):
        out[c // 4] += res.results[c]["out"].astype(np.float32)
    return out
